# revision 1
# baseline (speedup 1.0000x reference)
"""Trainium kernel for nn_Decoder: 30-step attention decoder with
adaptive softmax and greedy decode.

Self-contained: accepts FULL unsharded inputs, returns (preds [T,B] int32,
mean_loss fp32 scalar) exactly like the reference.

Strategy: the 30 decode steps are strictly sequential (greedy emb[pred]
feedback), so the step function is expressed in fp32 JAX and executed on
the NeuronCore backend as one fused program (lax.scan). Ops the Neuron
compiler rejects (multi-operand reduces from argmax / take_along_axis)
are rewritten to single-operand reduce forms. Falls back to CPU execution
if no accelerator is present.
"""
import numpy as np

# Hardcoded problem dims (from the problem spec)
S, B, H, E, V, T = 50, 64, 1024, 512, 50000, 30
C1, C2 = 3000, 20000
D1, D2 = 256, 128
PAD = 0
LN_EPS = 1e-5
BN_EPS = 1e-5


def _pick_device():
    import jax
    devs = jax.devices()
    for d in devs:
        if d.platform != "cpu":
            return d
    return devs[0]


def _build_step(static):
    import jax
    import jax.numpy as jnp

    (ctx, mask, emb, Wx0, bx0, gx0, Wh0, bh0, gh0,
     Wx1, bx1, gx1, Wh1, bh1, gh1, Wx2, bx2, gx2, Wh2, bh2, gh2,
     Wa, Wf, bf, bn_g, bn_b, Whead, bhead, P1, O1, P2, O2) = static
    bn_scale = np.float32(1.0 / np.sqrt(1.0 + BN_EPS))
    iota_v = jnp.arange(V, dtype=jnp.int32)

    def _ln(x, g):
        m = x.mean(-1, keepdims=True)
        v = ((x - m) ** 2).mean(-1, keepdims=True)
        return g * (x - m) / jnp.sqrt(v + LN_EPS)

    def _lngru(x, h, Wx, bx, gx, Wh, bh, gh):
        gi = _ln(x @ Wx + bx, gx)
        gh_ = _ln(h @ Wh + bh, gh)
        ir, iz, in_ = jnp.split(gi, 3, axis=-1)
        hr, hz, hn = jnp.split(gh_, 3, axis=-1)
        r = jax.nn.sigmoid(ir + hr)
        z = jax.nn.sigmoid(iz + hz)
        n = jnp.tanh(in_ + r * hn)
        return (1.0 - z) * n + z * h

    def _lsm(x):
        m = jax.lax.stop_gradient(x.max(-1, keepdims=True))
        s = x - m
        return s - jnp.log(jnp.exp(s).sum(-1, keepdims=True))

    def step(carry, tgt_t):
        h, prev_y = carry
        h = _lngru(prev_y, h, Wx0, bx0, gx0, Wh0, bh0, gh0)
        scores = jnp.einsum('bk,sbk->bs', h @ Wa, ctx) + mask
        sm = scores - jax.lax.stop_gradient(scores.max(1, keepdims=True))
        e = jnp.exp(sm)
        attn = e / e.sum(1, keepdims=True)
        context = jnp.einsum('bs,sbk->bk', attn, ctx)
        h = _lngru(context, h, Wx1, bx1, gx1, Wh1, bh1, gh1)
        h = _lngru(jnp.zeros((B, 1), jnp.float32), h,
                   Wx2, bx2, gx2, Wh2, bh2, gh2)
        y = jnp.tanh(bn_g * ((h @ Wf + bf) * bn_scale) + bn_b)
        head = _lsm(y @ Whead + bhead)
        t1 = _lsm((y @ P1) @ O1)
        t2 = _lsm((y @ P2) @ O2)
        lp = jnp.concatenate([head[:, :C1],
                              head[:, C1:C1 + 1] + t1,
                              head[:, C1 + 1:C1 + 2] + t2], axis=1)
        # loss: gather target log-prob via one-hot dot (avoids
        # take_along_axis gather patterns the compiler rejects)
        tgt_lp = jnp.where(iota_v[None, :] == tgt_t[:, None], lp, 0.0
                           ).sum(axis=1)
        loss_t = -tgt_lp.mean()
        # argmax without a (value, index) multi-operand reduce:
        m = lp.max(axis=1, keepdims=True)
        pred = jnp.where(lp == m, iota_v[None, :], V).min(axis=1)
        pred = pred.astype(jnp.int32)
        return (h, emb[pred]), (pred, loss_t)

    return step


def kernel(ctx, hidden, inputs, targets, target_len, emb,
           Wx0, bx0, gx0, Wh0, bh0, gh0,
           Wx1, bx1, gx1, Wh1, bh1, gh1,
           Wx2, bx2, gx2, Wh2, bh2, gh2,
           Wa, Wf, bf, bn_g, bn_b,
           Whead, bhead, P1, O1, P2, O2):
    import jax
    import jax.numpy as jnp

    tl = int(np.asarray(target_len))
    dev = _pick_device()

    f32 = lambda a: np.asarray(a, dtype=np.float32)
    mask = np.where(np.asarray(inputs) == PAD, -np.inf, 0.0).astype(np.float32)

    def run_on(device):
        with jax.default_device(device):
            static = tuple(jnp.asarray(a) for a in
                           (f32(ctx), mask, f32(emb),
                            f32(Wx0), f32(bx0), f32(gx0),
                            f32(Wh0), f32(bh0), f32(gh0),
                            f32(Wx1), f32(bx1), f32(gx1),
                            f32(Wh1), f32(bh1), f32(gh1),
                            f32(Wx2), f32(bx2), f32(gx2),
                            f32(Wh2), f32(bh2), f32(gh2),
                            f32(Wa), f32(Wf), f32(bf),
                            f32(bn_g), f32(bn_b),
                            f32(Whead), f32(bhead),
                            f32(P1), f32(O1), f32(P2), f32(O2)))
            step = _build_step(static)
            tgts = jnp.asarray(np.asarray(targets).T[:tl].astype(np.int32))
            init = (jnp.asarray(f32(hidden)),
                    jnp.zeros((B, E), jnp.float32))

            @jax.jit
            def run(init, tgts):
                _, (preds, losses) = jax.lax.scan(step, init, tgts)
                return preds, losses.sum() / np.float32(tl)

            preds, loss = run(init, tgts)
            preds = np.asarray(jax.device_get(preds)).astype(np.int32)
            loss = np.float32(jax.device_get(loss))
            return preds, loss

    try:
        return run_on(dev)
    except Exception:
        # accelerator path failed (compiler/runtime) — fall back to CPU
        cpu = jax.local_devices(backend="cpu")[0]
        return run_on(cpu)


if __name__ == "__main__":
    pass
